# revision 6
# baseline (speedup 1.0000x reference)
"""Trainium2 Bass kernel for multi-head attention (B=2, S=2048, D=1024, H=16).

Sharding: data-parallel over query rows. Core c handles batch b=c//4 and
query rows [512*(c%4), 512*(c%4+1)). Each core computes K/V projections for
all heads over the full sequence (duplicated across the 4 cores sharing a
batch), Q projection for its 512 rows, attention, and the output projection
for its rows. No cross-core communication.

Layouts (all chosen so the contraction dim lands on SBUF partitions and no
on-device transposes are needed):
  xT   [8,128,2048]  x[b] transposed (d on partitions), s-axis rolled so this
                     core's q-block sits at columns 0:512
  kT   per 2-head group [128, 2048]: partitions = (head parity)*64 + dh
  v    per s-chunk [128, 4, 65]: v for 4 heads + denominator column
  scores^T [s, q] so the attn@v contraction needs no transpose; softmax
  denominator comes from the extra column of v (M=65 matmul output row 64).

Padding mask: V rows (and the denominator column) are multiplied by the 0/1
key mask, so masked keys contribute exactly 0 to both the numerator and the
softmax denominator — identical to the reference's -1e9 score masking, and
it keeps the exp activation bias-free so two score chunks share one
[128, 1024] exp op. Softmax skips max-subtraction (scores are ~N(0,1) after
the 1/8 scale; exp cannot overflow fp32).

All matmuls use float32r (TF32-like, full PE rate at N>=256; inputs are
pre-rounded on the host) with K=128 (scores use K=128 with the unused
head-half of q zeroed so the PE never switches tiling modes).
"""

import os
import sys

sys.path.insert(0, "/opt/trn_rl_repo")

import numpy as np

B, S, D, H, DH = 2, 2048, 1024, 16, 64
NCORES = 8
CPB = NCORES // B       # cores per batch
QB = S // CPB           # 512 query rows per core
P = 128
DCH = D // P            # 8 contraction chunks
SC = S // P             # 16 s-chunks
NEG = -1e9

_compiled = {}
LAST_RESULTS = None
ABLATE = set()   # debug: {"kv1","exp_copy","b1","c1"} cripple phases for HW bisection
UNROLL = 1       # debug: repeat the whole body N times inside one NEFF


def _build_program():
    import concourse.bass as bass
    import concourse.mybir as mybir
    import concourse.tile as tile
    from concourse import bacc

    f32 = mybir.dt.float32
    f32r = mybir.dt.float32r
    AF = mybir.ActivationFunctionType
    OP = mybir.AluOpType


    nc = bacc.Bacc(
        "TRN2", target_bir_lowering=False, debug=False,
        num_devices=NCORES,
    )

    xT = nc.dram_tensor("xT", [DCH, P, S], f32r, kind="ExternalInput")
    wq = nc.dram_tensor("wq", [H // 2, P, DCH, P], f32r, kind="ExternalInput")
    wk = nc.dram_tensor("wk", [H // 2, P, DCH, P], f32r, kind="ExternalInput")
    wv = nc.dram_tensor("wv", [H // 4, P, DCH, 256], f32r, kind="ExternalInput")
    woT = nc.dram_tensor("woT", [DCH, P, D], f32r, kind="ExternalInput")
    bq = nc.dram_tensor("bq", [P, H // 2], f32, kind="ExternalInput")
    bk = nc.dram_tensor("bk", [P, H // 2], f32, kind="ExternalInput")
    bv = nc.dram_tensor("bv", [1, D], f32, kind="ExternalInput")
    bo = nc.dram_tensor("bo", [1, D], f32, kind="ExternalInput")
    maskT = nc.dram_tensor("maskT", [P, SC], f32, kind="ExternalInput")
    out = nc.dram_tensor("out", [QB, D], f32, kind="ExternalOutput")

    with tile.TileContext(nc) as tc:
        with (
            tc.tile_pool(name="const", bufs=1) as constp,
            tc.tile_pool(name="big", bufs=DCH) as bigp,
            tc.tile_pool(name="wo", bufs=DCH) as wobigp,
            tc.tile_pool(name="w", bufs=2) as wpool,
            tc.tile_pool(name="kt", bufs=2) as ktpool,
            tc.tile_pool(name="va", bufs=SC) as vpool,
            tc.tile_pool(name="qtz", bufs=4) as qpool,
            tc.tile_pool(name="pt", bufs=4) as ptpool,
            tc.tile_pool(name="cat", bufs=1) as catp,
            tc.tile_pool(name="rr", bufs=2) as rpool,
            tc.tile_pool(name="osb", bufs=2) as outp,
            tc.tile_pool(name="pp", bufs=2, space="PSUM") as pp,
            tc.tile_pool(name="psc", bufs=2, space="PSUM") as psc,
            tc.tile_pool(name="po", bufs=2, space="PSUM") as pop,
        ):
            # ---- constants
            bq_sb = constp.tile([P, H // 2], f32, tag="bq")
            nc.sync.dma_start(out=bq_sb[:], in_=bq[:])
            bk_sb = constp.tile([P, H // 2], f32, tag="bk")
            nc.sync.dma_start(out=bk_sb[:], in_=bk[:])
            mask_sb = constp.tile([P, SC], f32, tag="mask")
            nc.sync.dma_start(out=mask_sb[:], in_=maskT[:])
            bv_src = constp.tile([1, D], f32, tag="bvs")
            nc.sync.dma_start(out=bv_src[:], in_=bv[:])
            bo_src = constp.tile([1, D], f32, tag="bos")
            nc.sync.dma_start(out=bo_src[:], in_=bo[:])
            bv_rep = constp.tile([P, D], f32, tag="bvr")
            nc.gpsimd.partition_broadcast(bv_rep[:], bv_src[:])
            bo_rep = constp.tile([P, D], f32, tag="bor")
            nc.gpsimd.partition_broadcast(bo_rep[:], bo_src[:])

            for rep in range(UNROLL):
              concat = catp.tile([P, DCH, QB], f32r, tag="cat",
                                 name=f"cat{rep}")

              # ---- x^T resident in SBUF (8 chunks of [128, 2048])
              xt = []
              for d in range(DCH):
                  t = bigp.tile([P, S], f32r, tag="big", name=f"xt{rep}_{d}")
                  nc.sync.dma_start(out=t[:], in_=xT[d])
                  xt.append(t)

              NW = 4          # waves
              HPW = H // NW   # heads per wave

              for wave in range(NW):
                  groups = [2 * wave, 2 * wave + 1]
                  # ---- A: kT projection (2-head groups, output [2*64 dh, s])
                  kt = []
                  for gl, g in enumerate(groups):
                      wk_t = wpool.tile([P, DCH, P], f32r, tag="wk")
                      nc.sync.dma_start(out=wk_t[:], in_=wk[g])
                      ktile = ktpool.tile([P, S], f32r, tag="kt")
                      DR = 1 if "kv1" in ABLATE else DCH
                      for sb in range(4):
                          ps = pp.tile([P, 512], f32, tag="pp")
                          for d in range(DR):
                              nc.tensor.matmul(
                                  ps[:],
                                  wk_t[:, d, :],
                                  xt[d][:, sb * 512:(sb + 1) * 512],
                                  start=(d == 0),
                                  stop=(d == DR - 1),
                              )
                          nc.vector.tensor_scalar_add(
                              ktile[:, sb * 512:(sb + 1) * 512], ps[:],
                              bk_sb[:, g:g + 1],
                          )
                      kt.append(ktile)

                  # ---- A: v projection (4 heads at once, natural [s, 4*64])
                  wv_t = wpool.tile([P, DCH, 256], f32r, tag="wv")
                  nc.sync.dma_start(out=wv_t[:], in_=wv[wave])
                  va = []
                  for sc in range(SC):
                      vt = vpool.tile([P, HPW, 65], f32r, tag="va")
                      ps = pp.tile([P, 512], f32, tag="pp",
                                   name=f"vps_{wave}_{sc}")[:, 0:256]
                      DR = 1 if "kv1" in ABLATE else DCH
                      for d in range(DR):
                          nc.tensor.matmul(
                              ps[:],
                              xt[d][:, sc * P:(sc + 1) * P],
                              wv_t[:, d, :],
                              start=(d == 0),
                              stop=(d == DR - 1),
                          )
                      ps_r = ps.rearrange("p (h e) -> p h e", e=64)
                      nc.vector.tensor_tensor(
                          vt[:, :, 0:64],
                          ps_r,
                          bv_rep[:, wave * 256:(wave + 1) * 256].rearrange(
                              "p (h e) -> p h e", e=64),
                          OP.add,
                      )
                      # zero out masked key rows: masked s contributes 0 to
                      # both numerator and denominator (same as -1e9 scores)
                      nc.vector.tensor_scalar(
                          vt[:, :, 0:64], vt[:, :, 0:64],
                          mask_sb[:, sc:sc + 1], None, OP.mult,
                      )
                      # denominator column = mask (1 for valid, 0 for padded)
                      nc.vector.tensor_scalar(
                          vt[:, :, 64:65], ps_r[:, :, 0:1], 0.0,
                          mask_sb[:, sc:sc + 1], OP.mult, OP.add,
                      )
                      va.append(vt)

                  # ---- A: q projection for this wave's groups. One [128, 512]
                  # tile per group: rows 0:64 = even head, 64:128 = odd head
                  # (bias added, no zeroing -- the scores matmuls are row-tiled
                  # at K=64 so the other half is never read).
                  qtz = []
                  for gl, g in enumerate(groups):
                      wq_t = wpool.tile([P, DCH, P], f32r, tag="wq")
                      nc.sync.dma_start(out=wq_t[:], in_=wq[g])
                      ps = pp.tile([P, 512], f32, tag="pp")
                      for d in range(DCH):
                          nc.tensor.matmul(
                              ps[:],
                              wq_t[:, d, :],
                              xt[d][:, 0:QB],
                              start=(d == 0),
                              stop=(d == DCH - 1),
                          )
                      qz = qpool.tile([P, QB], f32r, tag="qtz")
                      nc.vector.tensor_scalar_add(
                          qz[:], ps[:], bq_sb[:, g:g + 1],
                      )
                      qtz.append(qz)

                  # ---- B: attention, one head PAIR at a time. The two score
                  # matmuls per s-chunk are row-tiled (K=64 at row positions 0
                  # and 64) so they run concurrently on the PE -- one N=512
                  # slot computes both heads' scores for the chunk.
                  for gl in range(2):
                      hl0, hl1 = 2 * gl, 2 * gl + 1
                      po0 = pop.tile([P, QB], f32, tag="po")
                      po1 = pop.tile([P, QB], f32, tag="po")
                      pts = {}

                      def emit_scores(sc):
                          # both heads' scores for chunk sc -> [128, 1024]
                          # psum (2 banks), one exp over both
                          sps = psc.tile([P, 2, QB], f32, tag="ps")
                          nc.tensor.matmul(
                              sps[:, 0, :],
                              kt[gl][0:64, sc * P:(sc + 1) * P],
                              qtz[gl][0:64, :],
                              start=True,
                              stop=True,
                          )
                          nc.tensor.matmul(
                              sps[:, 1, :],
                              kt[gl][64:P, sc * P:(sc + 1) * P],
                              qtz[gl][64:P, :],
                              start=True,
                              stop=True,
                          )
                          pt = ptpool.tile([P, 2, QB], f32r, tag="pt")
                          if "exp_copy" in ABLATE:
                              nc.vector.tensor_scalar(
                                  pt[:], sps[:], 0.125, None, OP.mult)
                          else:
                              nc.scalar.activation(
                                  pt[:], sps[:], AF.Exp,
                                  bias=0.0, scale=0.125,
                              )
                          pts[sc] = pt

                      def emit_o(sc):
                          pt = pts.pop(sc)
                          nc.tensor.matmul(
                              po0[0:65, :],
                              va[sc][:, hl0, :],
                              pt[:, 0, :],
                              start=(sc == 0),
                              stop=(sc == SC - 1),
                          )
                          nc.tensor.matmul(
                              po1[0:65, :],
                              va[sc][:, hl1, :],
                              pt[:, 1, :],
                              start=(sc == 0),
                              stop=(sc == SC - 1),
                          )

                      if "b1" in ABLATE:
                          emit_scores(0)
                          pt = pts.pop(0)
                          nc.tensor.matmul(
                              po0[0:65, :], va[0][:, hl0, :], pt[:, 0, :],
                              start=True, stop=True)
                          nc.tensor.matmul(
                              po1[0:65, :], va[0][:, hl1, :], pt[:, 1, :],
                              start=True, stop=True)
                      else:
                          emit_scores(0)
                          emit_scores(1)
                          for sc in range(2, SC):
                              emit_o(sc - 2)
                              emit_scores(sc)
                          emit_o(SC - 2)
                          emit_o(SC - 1)

                      # normalize: row 64 of po is the softmax denominator
                      cslot = wave * 2 + gl
                      for par, po_t in ((0, po0), (1, po1)):
                          den = rpool.tile([65, QB], f32, tag="den")
                          nc.vector.reciprocal(den[64:65, :], po_t[64:65, :])
                          # partition_broadcast requires a base-0 input on HW
                          den0 = rpool.tile([1, QB], f32, tag="den0")
                          nc.sync.dma_start(out=den0[:], in_=den[64:65, :])
                          rep = rpool.tile([P, QB], f32, tag="rep")
                          nc.gpsimd.partition_broadcast(rep[:], den0[0:1, :])
                          if par == 0:
                              nc.vector.tensor_tensor(
                                  concat[0:64, cslot, :], po_t[0:64, :],
                                  rep[0:64, :], OP.mult,
                              )
                          else:
                              tmp = rpool.tile([64, QB], f32r, tag="tmp")
                              nc.vector.tensor_tensor(
                                  tmp[:], po_t[0:64, :], rep[0:64, :], OP.mult,
                              )
                              nc.sync.dma_start(
                                  out=concat[64:P, cslot, :], in_=tmp[:],
                              )

              # ---- C: output projection (contraction over h*dh in 8 chunks)
              wo_sb = []
              for c in range(DCH):
                  t = wobigp.tile([P, D], f32r, tag="wo")
                  nc.sync.dma_start(out=t[:], in_=woT[c])
                  wo_sb.append(t)
              for qt_i in range(QB // P):
                  for eb in range(2):
                      ps = pp.tile([P, 512], f32, tag="pp")
                      CR = 1 if "c1" in ABLATE else DCH
                      for c in range(CR):
                          nc.tensor.matmul(
                              ps[:],
                              concat[:, c, qt_i * P:(qt_i + 1) * P],
                              wo_sb[c][:, eb * 512:(eb + 1) * 512],
                              start=(c == 0),
                              stop=(c == CR - 1),
                          )
                      osb = outp.tile([P, 512], f32, tag="osb")
                      nc.vector.tensor_tensor(
                          osb[:], ps[:], bo_rep[:, eb * 512:(eb + 1) * 512],
                          OP.add,
                      )
                      nc.sync.dma_start(
                          out=out[qt_i * P:(qt_i + 1) * P,
                                  eb * 512:(eb + 1) * 512],
                          in_=osb[:],
                      )

    nc.compile()
    nc.finalize()
    return nc


def _round_fp32r(a):
    """Round fp32 values to fp32r (TF32-like, 11-bit mantissa, RNE)."""
    u = np.ascontiguousarray(a, dtype=np.float32).view(np.uint32).astype(np.uint64)
    r = ((u + 0x7FF + ((u >> 12) & 1)) & 0xFFFFF000).astype(np.uint32)
    return r.view(np.float32).reshape(a.shape)


def prep_inputs(x, pad_mask, wq, wk, wv, bq, bk, bv, wo, bo):
    """Build per-core input maps (host-side shard + layout prep)."""
    x = np.ascontiguousarray(np.asarray(x, dtype=np.float32))
    pad_mask = np.asarray(pad_mask)
    wq = np.asarray(wq, dtype=np.float32)
    wk = np.asarray(wk, dtype=np.float32)
    wv = np.asarray(wv, dtype=np.float32)
    bq = np.asarray(bq, dtype=np.float32)
    bk = np.asarray(bk, dtype=np.float32)
    bv = np.asarray(bv, dtype=np.float32)
    wo = np.asarray(wo, dtype=np.float32)
    bo = np.asarray(bo, dtype=np.float32)

    # weights: [H, D, DH] -> [d, h*dh] (h-major columns)
    def stack_groups(w, gsz):
        ws = np.ascontiguousarray(w.transpose(1, 0, 2).reshape(D, D))
        # -> [group, di, do, gsz*DH]
        m = gsz * DH
        arr = ws.reshape(DCH, P, H // gsz, m).transpose(2, 1, 0, 3)
        return np.ascontiguousarray(arr)

    wq_dev = _round_fp32r(stack_groups(wq, 2))
    wk_dev = _round_fp32r(stack_groups(wk, 2))
    wv_dev = _round_fp32r(stack_groups(wv, 4))
    woT_dev = _round_fp32r(np.ascontiguousarray(wo.T).reshape(DCH, P, D))
    bq_dev = np.ascontiguousarray(bq.reshape(H // 2, P).T)
    bk_dev = np.ascontiguousarray(bk.reshape(H // 2, P).T)
    bv_dev = np.ascontiguousarray(bv.reshape(1, D))
    bo_dev = np.ascontiguousarray(bo.reshape(1, D))

    in_maps = []
    for c in range(NCORES):
        b, qo = c // CPB, c % CPB
        # transpose + roll the s axis so this core's q rows are cols 0:QB
        xt = x[b].T  # [D, S]
        xt = np.roll(xt, -qo * QB, axis=1)
        xt_dev = _round_fp32r(np.ascontiguousarray(xt)).reshape(DCH, P, S)
        m01 = (pad_mask[b] != 0).astype(np.float32)
        m01 = np.roll(m01, -qo * QB)
        maskT_dev = np.ascontiguousarray(m01.reshape(SC, P).T)
        in_maps.append({
            "xT": xt_dev, "wq": wq_dev, "wk": wk_dev, "wv": wv_dev,
            "woT": woT_dev, "bq": bq_dev, "bk": bk_dev, "bv": bv_dev,
            "bo": bo_dev, "maskT": maskT_dev,
        })
    return in_maps


def kernel(**inputs):
    global LAST_RESULTS
    from concourse.bass_utils import run_bass_kernel_spmd

    if "nc" not in _compiled:
        _compiled["nc"] = _build_program()
    nc = _compiled["nc"]

    in_maps = prep_inputs(**inputs)
    res = run_bass_kernel_spmd(
        nc, in_maps, list(range(NCORES)),
        trace=bool(os.environ.get("BASS_TRACE")),
    )
    LAST_RESULTS = res

    out = np.empty((B, S, D), dtype=np.float32)
    for c in range(NCORES):
        b, qo = c // CPB, c % CPB
        out[b, qo * QB:(qo + 1) * QB, :] = res.results[c]["out"]
    return out



# revision 29
# speedup vs baseline: 1.5197x; 1.5197x over previous
"""Trainium2 Bass kernel for multi-head attention (B=2, S=2048, D=1024, H=16).

Sharding: data-parallel over query rows. Core c handles batch b=c//4 and
query rows [512*(c%4), 512*(c%4+1)). Each core computes K/V projections for
all heads over the full sequence (duplicated across the 4 cores sharing a
batch), Q projection for its 512 rows, attention, and the output projection
for its rows. No cross-core communication.

Layouts (all chosen so the contraction dim lands on SBUF partitions and no
on-device transposes are needed):
  xT   [8,128,2048]  x[b] transposed (d on partitions), s-axis rolled so this
                     core's q-block sits at columns 0:512
  kT   per 2-head group [128, 2048]: partitions = (head parity)*64 + dh
  v    per s-chunk [128, 4, 65]: v for 4 heads + denominator column
  scores^T [s, q] so the attn@v contraction needs no transpose; softmax
  denominator comes from the extra column of v (M=65 matmul output row 64).

Padding mask: V rows (and the denominator column) are multiplied by the 0/1
key mask, so masked keys contribute exactly 0 to both the numerator and the
softmax denominator — identical to the reference's -1e9 score masking, and
it keeps the exp activation bias-free so two score chunks share one
[128, 1024] exp op. Softmax skips max-subtraction (scores are ~N(0,1) after
the 1/8 scale; exp cannot overflow fp32).

All matmuls use float32r (TF32-like, full PE rate at N>=256; inputs are
pre-rounded on the host) with K=128 (scores use K=128 with the unused
head-half of q zeroed so the PE never switches tiling modes).
"""

import os
import sys

sys.path.insert(0, "/opt/trn_rl_repo")

import numpy as np

B, S, D, H, DH = 2, 2048, 1024, 16, 64
NCORES = 8
CPB = NCORES // B       # cores per batch
QB = S // CPB           # 512 query rows per core
P = 128
DCH = D // P            # 8 contraction chunks
SC = S // P             # 16 s-chunks
NEG = -1e9

_compiled = {}
LAST_RESULTS = None
ABLATE = set()   # debug: {"kv1","exp_copy","b1","c1"} cripple phases for HW bisection
UNROLL = 1       # debug: repeat the whole body N times inside one NEFF


def _build_program():
    import concourse.bass as bass
    import concourse.mybir as mybir
    import concourse.tile as tile
    from concourse import bacc

    f32 = mybir.dt.float32
    f32r = mybir.dt.float32r
    bf16 = mybir.dt.bfloat16
    AF = mybir.ActivationFunctionType
    OP = mybir.AluOpType


    nc = bacc.Bacc(
        "TRN2", target_bir_lowering=False, debug=False,
        num_devices=NCORES,
    )

    xT = nc.dram_tensor("xT", [DCH, P, S], bf16, kind="ExternalInput")
    wq = nc.dram_tensor("wq", [H // 2, P, DCH, P], bf16, kind="ExternalInput")
    wk = nc.dram_tensor("wk", [H // 2, P, DCH, P], bf16, kind="ExternalInput")
    wv = nc.dram_tensor("wv", [H // 4, P, DCH, 256], bf16, kind="ExternalInput")
    woT = nc.dram_tensor("woT", [DCH, P, D], bf16, kind="ExternalInput")
    bq = nc.dram_tensor("bq", [P, H // 2], f32, kind="ExternalInput")
    bk = nc.dram_tensor("bk", [P, H // 2], f32, kind="ExternalInput")
    bv = nc.dram_tensor("bv", [1, D], f32, kind="ExternalInput")
    bo = nc.dram_tensor("bo", [1, D], f32, kind="ExternalInput")
    maskT = nc.dram_tensor("maskT", [P, SC], f32, kind="ExternalInput")
    oneC = nc.dram_tensor("oneC", [P, 1], bf16, kind="ExternalInput")
    out = nc.dram_tensor("out", [QB, D], f32, kind="ExternalOutput")

    with tile.TileContext(nc) as tc:
        with (
            tc.tile_pool(name="const", bufs=1) as constp,
            tc.tile_pool(name="big", bufs=DCH) as bigp,
            tc.tile_pool(name="wo", bufs=DCH) as wobigp,
            tc.tile_pool(name="w", bufs=2) as wpool,
            tc.tile_pool(name="kt", bufs=2) as ktpool,
            tc.tile_pool(name="va", bufs=SC) as vpool,
            tc.tile_pool(name="qtz", bufs=4) as qpool,
            tc.tile_pool(name="pt", bufs=4) as ptpool,
            tc.tile_pool(name="cat", bufs=1) as catp,
            tc.tile_pool(name="rr", bufs=1) as rpool,
            tc.tile_pool(name="osb", bufs=2) as outp,
            tc.tile_pool(name="pp", bufs=2, space="PSUM") as pp,
            tc.tile_pool(name="psc", bufs=2, space="PSUM") as psc,
            tc.tile_pool(name="po", bufs=2, space="PSUM") as pop,
        ):
            # ---- constants
            bq_sb = constp.tile([P, H // 2], f32, tag="bq")
            nc.sync.dma_start(out=bq_sb[:], in_=bq[:])
            bk_sb = constp.tile([P, H // 2], f32, tag="bk")
            nc.sync.dma_start(out=bk_sb[:], in_=bk[:])
            mask_sb = constp.tile([P, SC], f32, tag="mask")
            nc.sync.dma_start(out=mask_sb[:], in_=maskT[:])
            ones_sb = constp.tile([P, 1], bf16, tag="ones")
            nc.sync.dma_start(out=ones_sb[:], in_=oneC[:])
            bv_src = constp.tile([1, D], f32, tag="bvs")
            nc.sync.dma_start(out=bv_src[:], in_=bv[:])
            bo_src = constp.tile([1, D], f32, tag="bos")
            nc.sync.dma_start(out=bo_src[:], in_=bo[:])
            bv_rep = constp.tile([P, D], f32, tag="bvr")
            nc.gpsimd.partition_broadcast(bv_rep[:], bv_src[:])
            bo_rep = constp.tile([P, D], f32, tag="bor")
            nc.gpsimd.partition_broadcast(bo_rep[:], bo_src[:])

            for rep in range(UNROLL):
              concat = catp.tile([P, DCH, QB], bf16, tag="cat",
                                 name=f"cat{rep}")

              # ---- x^T resident in SBUF (8 chunks of [128, 2048])
              xt = []
              for d in range(DCH):
                  t = bigp.tile([P, S], bf16, tag="big", name=f"xt{rep}_{d}")
                  nc.sync.dma_start(out=t[:], in_=xT[d])
                  xt.append(t)

              NW = 4          # waves
              HPW = H // NW   # heads per wave

              for wave in range(NW):
                  groups = [2 * wave, 2 * wave + 1]
                  # ---- A: kT projection (2-head groups, output [2*64 dh, s])
                  kt = []
                  for gl, g in enumerate(groups):
                      wk_t = wpool.tile([P, DCH, P], bf16, tag="wk")
                      nc.sync.dma_start(out=wk_t[:], in_=wk[g])
                      ktile = ktpool.tile([P, S], f32r, tag="kt")
                      DR = 1 if "kv1" in ABLATE else DCH
                      for sb in range(4):
                          ps = pp.tile([P, 512], f32, tag="pp")
                          for d in range(DR):
                              nc.tensor.matmul(
                                  ps[:],
                                  wk_t[:, d, :],
                                  xt[d][:, sb * 512:(sb + 1) * 512],
                                  start=(d == 0),
                                  stop=(d == DR - 1),
                              )
                          nc.vector.tensor_scalar_add(
                              ktile[:, sb * 512:(sb + 1) * 512], ps[:],
                              bk_sb[:, g:g + 1],
                          )
                      kt.append(ktile)

                  # ---- A: v projection (4 heads at once, natural [s, 4*64])
                  wv_t = wpool.tile([P, DCH, 256], bf16, tag="wv")
                  nc.sync.dma_start(out=wv_t[:], in_=wv[wave])
                  va = []
                  for sc in range(SC):
                      # masked keys are handled by the exp bias (-50 on masked
                      # partitions), so V needs no masking and no denominator
                      # column here.
                      vt = vpool.tile([P, HPW, 64], bf16, tag="va")
                      ps = pp.tile([P, 512], f32, tag="pp",
                                   name=f"vps_{wave}_{sc}")[:, 0:256]
                      DR = 1 if "kv1" in ABLATE else DCH
                      for d in range(DR):
                          nc.tensor.matmul(
                              ps[:],
                              xt[d][:, sc * P:(sc + 1) * P],
                              wv_t[:, d, :],
                              start=(d == 0),
                              stop=(d == DR - 1),
                          )
                      ps_r = ps.rearrange("p (h e) -> p h e", e=64)
                      nc.vector.tensor_tensor(
                          vt[:],
                          ps_r,
                          bv_rep[:, wave * 256:(wave + 1) * 256].rearrange(
                              "p (h e) -> p h e", e=64),
                          OP.add,
                      )
                      va.append(vt)

                  # ---- A: q projection for this wave's groups. One [128, 512]
                  # tile per group: rows 0:64 = even head, 64:128 = odd head
                  # (bias added, no zeroing -- the scores matmuls are row-tiled
                  # at K=64 so the other half is never read).
                  qtz = []
                  for gl, g in enumerate(groups):
                      wq_t = wpool.tile([P, DCH, P], bf16, tag="wq")
                      nc.sync.dma_start(out=wq_t[:], in_=wq[g])
                      ps = pp.tile([P, 512], f32, tag="pp")
                      for d in range(DCH):
                          nc.tensor.matmul(
                              ps[:],
                              wq_t[:, d, :],
                              xt[d][:, 0:QB],
                              start=(d == 0),
                              stop=(d == DCH - 1),
                          )
                      qz = qpool.tile([P, QB], f32r, tag="qtz")
                      nc.vector.tensor_scalar_add(
                          qz[:], ps[:], bq_sb[:, g:g + 1],
                      )
                      qtz.append(qz)

                  # ---- B: attention, one head PAIR at a time.
                  # scores: row-tiled (K=64 at row 0 / 64) -- one N=512 slot
                  #   computes both heads' scores for a chunk.
                  # AV: col-tiled (M=64 at col 0 / 64) -- one slot accumulates
                  #   both heads into one PSUM bank (h0 rows 0:64, h1 64:128).
                  # den: per chunk PAIR, four col-tiled M=1 matmuls with a
                  #   ones-column stationary -- one slot per 2 chunks covers
                  #   both heads' denominators.
                  for gl in range(2):
                      hl0, hl1 = 2 * gl, 2 * gl + 1
                      po_pair = pop.tile([P, QB], f32, tag="po")
                      den_ps = pop.tile([P, QB], f32, tag="po")
                      pts = {}

                      def emit_scores(sc):
                          # both heads' scores for chunk sc -> [128, 1024]
                          # psum (2 banks), one exp over both; key-pad mask is
                          # applied via the per-partition exp bias (0 / -50)
                          sps = psc.tile([P, 2, QB], f32, tag="ps")
                          nc.tensor.matmul(
                              sps[:, 0, :],
                              kt[gl][0:64, sc * P:(sc + 1) * P],
                              qtz[gl][0:64, :],
                              start=True,
                              stop=True,
                          )
                          nc.tensor.matmul(
                              sps[:, 1, :],
                              kt[gl][64:P, sc * P:(sc + 1) * P],
                              qtz[gl][64:P, :],
                              start=True,
                              stop=True,
                          )
                          pt = ptpool.tile([P, 2, QB], bf16, tag="pt")
                          if "exp_copy" in ABLATE:
                              nc.vector.tensor_scalar(
                                  pt[:], sps[:], 0.125, None, OP.mult)
                          else:
                              nc.scalar.activation(
                                  pt[:], sps[:], AF.Exp,
                                  bias=mask_sb[:, sc:sc + 1], scale=0.125,
                              )
                          pts[sc] = pt

                      def emit_av(sc):
                          pt = pts[sc]
                          nc.tensor.matmul(
                              po_pair[0:64, :],
                              va[sc][:, hl0, :],
                              pt[:, 0, :],
                              start=(sc == 0),
                              stop=(sc == SC - 1),
                          )
                          nc.tensor.matmul(
                              po_pair[64:P, :],
                              va[sc][:, hl1, :],
                              pt[:, 1, :],
                              start=(sc == 0),
                              stop=(sc == SC - 1),
                          )

                      def emit_den(sc):
                          # chunk pair (sc-1, sc): 4 M=1 col tiles at rows
                          # 0/32/64/96 of den_ps: [h0 even, h1 even, h0 odd,
                          # h1 odd]
                          pt_e, pt_o = pts.pop(sc - 1), pts.pop(sc)
                          pc = sc // 2
                          first, last = pc == 0, pc == SC // 2 - 1
                          for j, (ptx, h) in enumerate(
                              ((pt_e, 0), (pt_o, 0), (pt_e, 1), (pt_o, 1))
                          ):
                              r = (0, 64, 32, 96)[j]
                              nc.tensor.matmul(
                                  den_ps[r:r + 1, :],
                                  ones_sb[:],
                                  ptx[:, h, :],
                                  start=first,
                                  stop=last,
                                  tile_position=(0, r),
                              )

                      def emit_o(sc):
                          emit_av(sc)
                          if sc % 2 == 1:
                              emit_den(sc)

                      if "b1" in ABLATE:
                          emit_scores(0)
                          pt = pts.pop(0)
                          nc.tensor.matmul(
                              po_pair[0:64, :], va[0][:, hl0, :], pt[:, 0, :],
                              start=True, stop=True)
                          nc.tensor.matmul(
                              po_pair[64:P, :], va[0][:, hl1, :], pt[:, 1, :],
                              start=True, stop=True)
                      else:
                          emit_scores(0)
                          emit_scores(1)
                          for sc in range(2, SC):
                              emit_o(sc - 2)
                              emit_scores(sc)
                          emit_o(SC - 2)
                          emit_o(SC - 1)

                      # normalize: den_ps rows {0,64} = h0 even/odd partial
                      # denominators, {32,96} = h1. Collapse to one partition,
                      # one reciprocal + one broadcast for both heads.
                      cslot = wave * 2 + gl
                      dcp = rpool.tile([P, QB], f32, tag="dcp")
                      nc.vector.tensor_copy(dcp[0:97, :], den_ps[0:97, :])
                      # den4[h, j, :]: head h partials (even j=0, odd j=1)
                      den4 = rpool.tile([2, 2, QB], f32, tag="den4")
                      for (h, j), r in zip(
                          ((0, 0), (0, 1), (1, 0), (1, 1)), (0, 64, 32, 96)
                      ):
                          nc.sync.dma_start(
                              out=den4[h:h + 1, j, :], in_=dcp[r:r + 1, :])
                      dsum = rpool.tile([2, QB], f32, tag="dsum")
                      nc.vector.tensor_tensor(
                          dsum[:], den4[:, 0, :], den4[:, 1, :], OP.add)
                      dinv = rpool.tile([1, 2, QB], f32, tag="dinv")
                      nc.sync.dma_start(out=dinv[:], in_=dsum[:])
                      nc.vector.reciprocal(dinv[:], dinv[:])
                      rep = rpool.tile([P, 2, QB], f32, tag="rep")
                      nc.gpsimd.partition_broadcast(rep[:], dinv[0:1, :, :])
                      nc.vector.tensor_tensor(
                          concat[0:64, cslot, :], po_pair[0:64, :],
                          rep[0:64, 0, :], OP.mult,
                      )
                      nc.vector.tensor_tensor(
                          concat[64:P, cslot, :], po_pair[64:P, :],
                          rep[64:P, 1, :], OP.mult,
                      )

              # ---- C: output projection (contraction over h*dh in 8 chunks)
              wo_sb = []
              for c in range(DCH):
                  t = wobigp.tile([P, D], bf16, tag="wo")
                  nc.sync.dma_start(out=t[:], in_=woT[c])
                  wo_sb.append(t)
              for qt_i in range(QB // P):
                  for eb in range(2):
                      ps = pp.tile([P, 512], f32, tag="pp")
                      CR = 1 if "c1" in ABLATE else DCH
                      for c in range(CR):
                          nc.tensor.matmul(
                              ps[:],
                              concat[:, c, qt_i * P:(qt_i + 1) * P],
                              wo_sb[c][:, eb * 512:(eb + 1) * 512],
                              start=(c == 0),
                              stop=(c == CR - 1),
                          )
                      osb = outp.tile([P, 512], f32, tag="osb")
                      nc.vector.tensor_tensor(
                          osb[:], ps[:], bo_rep[:, eb * 512:(eb + 1) * 512],
                          OP.add,
                      )
                      nc.sync.dma_start(
                          out=out[qt_i * P:(qt_i + 1) * P,
                                  eb * 512:(eb + 1) * 512],
                          in_=osb[:],
                      )

    nc.compile()
    nc.finalize()
    return nc


def _round_fp32r(a):
    """Round fp32 values to fp32r (TF32-like, 11-bit mantissa, RNE)."""
    u = np.ascontiguousarray(a, dtype=np.float32).view(np.uint32).astype(np.uint64)
    r = ((u + 0x7FF + ((u >> 12) & 1)) & 0xFFFFF000).astype(np.uint32)
    return r.view(np.float32).reshape(a.shape)


def prep_inputs(x, pad_mask, wq, wk, wv, bq, bk, bv, wo, bo):
    """Build per-core input maps (host-side shard + layout prep)."""
    import ml_dtypes

    bf16 = ml_dtypes.bfloat16
    x = np.ascontiguousarray(np.asarray(x, dtype=np.float32))
    pad_mask = np.asarray(pad_mask)
    wq = np.asarray(wq, dtype=np.float32)
    wk = np.asarray(wk, dtype=np.float32)
    wv = np.asarray(wv, dtype=np.float32)
    bq = np.asarray(bq, dtype=np.float32)
    bk = np.asarray(bk, dtype=np.float32)
    bv = np.asarray(bv, dtype=np.float32)
    wo = np.asarray(wo, dtype=np.float32)
    bo = np.asarray(bo, dtype=np.float32)

    # weights: [H, D, DH] -> [d, h*dh] (h-major columns)
    def stack_groups(w, gsz):
        ws = np.ascontiguousarray(w.transpose(1, 0, 2).reshape(D, D))
        # -> [group, di, do, gsz*DH]
        m = gsz * DH
        arr = ws.reshape(DCH, P, H // gsz, m).transpose(2, 1, 0, 3)
        return np.ascontiguousarray(arr)

    wq_dev = stack_groups(wq, 2).astype(bf16)
    wk_dev = stack_groups(wk, 2).astype(bf16)
    wv_dev = stack_groups(wv, 4).astype(bf16)
    woT_dev = np.ascontiguousarray(wo.T).reshape(DCH, P, D).astype(bf16)
    bq_dev = np.ascontiguousarray(bq.reshape(H // 2, P).T)
    bk_dev = np.ascontiguousarray(bk.reshape(H // 2, P).T)
    bv_dev = np.ascontiguousarray(bv.reshape(1, D))
    bo_dev = np.ascontiguousarray(bo.reshape(1, D))

    in_maps = []
    for c in range(NCORES):
        b, qo = c // CPB, c % CPB
        # transpose + roll the s axis so this core's q rows are cols 0:QB
        xt = x[b].T  # [D, S]
        xt = np.roll(xt, -qo * QB, axis=1)
        xt_dev = np.ascontiguousarray(xt).astype(bf16).reshape(DCH, P, S)
        # exp-bias mask: 0 for valid keys, -50 for padded (exp -> ~0)
        mb = np.where(pad_mask[b] != 0, 0.0, -50.0).astype(np.float32)
        mb = np.roll(mb, -qo * QB)
        maskT_dev = np.ascontiguousarray(mb.reshape(SC, P).T)
        in_maps.append({
            "xT": xt_dev, "wq": wq_dev, "wk": wk_dev, "wv": wv_dev,
            "woT": woT_dev, "bq": bq_dev, "bk": bk_dev, "bv": bv_dev,
            "bo": bo_dev, "maskT": maskT_dev,
            "oneC": np.ones((P, 1), dtype=bf16),
        })
    return in_maps


def kernel(**inputs):
    global LAST_RESULTS
    from concourse.bass_utils import run_bass_kernel_spmd

    if "nc" not in _compiled:
        _compiled["nc"] = _build_program()
    nc = _compiled["nc"]

    in_maps = prep_inputs(**inputs)
    res = run_bass_kernel_spmd(
        nc, in_maps, list(range(NCORES)),
        trace=bool(os.environ.get("BASS_TRACE")),
    )
    LAST_RESULTS = res

    out = np.empty((B, S, D), dtype=np.float32)
    for c in range(NCORES):
        b, qo = c // CPB, c % CPB
        out[b, qo * QB:(qo + 1) * QB, :] = res.results[c]["out"]
    return out



# revision 35
# speedup vs baseline: 1.5806x; 1.0401x over previous
"""Trainium2 Bass kernel for multi-head attention (B=2, S=2048, D=1024, H=16).

Sharding: data-parallel over query rows. Core c handles batch b=c//4 and
query rows [512*(c%4), 512*(c%4+1)). Each core computes K/V projections for
all heads over the full sequence (duplicated across the 4 cores sharing a
batch), Q projection for its 512 rows, attention, and the output projection
for its rows. No cross-core communication.

Layouts (all chosen so the contraction dim lands on SBUF partitions and no
on-device transposes are needed):
  xT   [8,128,2048]  x[b] transposed (d on partitions), s-axis rolled so this
                     core's q-block sits at columns 0:512
  kT   per 2-head group [128, 2048]: partitions = (head parity)*64 + dh
  v    per s-chunk [128, 4, 65]: v for 4 heads + denominator column
  scores^T [s, q] so the attn@v contraction needs no transpose; softmax
  denominator comes from the extra column of v (M=65 matmul output row 64).

Padding mask: V rows (and the denominator column) are multiplied by the 0/1
key mask, so masked keys contribute exactly 0 to both the numerator and the
softmax denominator — identical to the reference's -1e9 score masking, and
it keeps the exp activation bias-free so two score chunks share one
[128, 1024] exp op. Softmax skips max-subtraction (scores are ~N(0,1) after
the 1/8 scale; exp cannot overflow fp32).

All matmuls use float32r (TF32-like, full PE rate at N>=256; inputs are
pre-rounded on the host) with K=128 (scores use K=128 with the unused
head-half of q zeroed so the PE never switches tiling modes).
"""

import os
import sys

sys.path.insert(0, "/opt/trn_rl_repo")

import numpy as np

B, S, D, H, DH = 2, 2048, 1024, 16, 64
NCORES = 8
CPB = NCORES // B       # cores per batch
QB = S // CPB           # 512 query rows per core
P = 128
DCH = D // P            # 8 contraction chunks
SC = S // P             # 16 s-chunks
NEG = -1e9

_compiled = {}
LAST_RESULTS = None
ABLATE = set()   # debug: {"kv1","exp_copy","b1","c1"} cripple phases for HW bisection
UNROLL = 1       # debug: repeat the whole body N times inside one NEFF


def _build_program():
    import concourse.bass as bass
    import concourse.mybir as mybir
    import concourse.tile as tile
    from concourse import bacc

    f32 = mybir.dt.float32
    f32r = mybir.dt.float32r
    bf16 = mybir.dt.bfloat16
    AF = mybir.ActivationFunctionType
    OP = mybir.AluOpType


    nc = bacc.Bacc(
        "TRN2", target_bir_lowering=False, debug=False,
        num_devices=NCORES,
    )

    xT = nc.dram_tensor("xT", [DCH, P, S], bf16, kind="ExternalInput")
    wq = nc.dram_tensor("wq", [H // 2, P, DCH, P], bf16, kind="ExternalInput")
    wk = nc.dram_tensor("wk", [H // 2, P, DCH, P], bf16, kind="ExternalInput")
    wv = nc.dram_tensor("wv", [H // 8, P, DCH, 512], bf16, kind="ExternalInput")
    woT = nc.dram_tensor("woT", [DCH, P, D], bf16, kind="ExternalInput")
    bq = nc.dram_tensor("bq", [P, H // 2], f32, kind="ExternalInput")
    bk = nc.dram_tensor("bk", [P, H // 2], f32, kind="ExternalInput")
    bv = nc.dram_tensor("bv", [1, D], f32, kind="ExternalInput")
    bo = nc.dram_tensor("bo", [1, D], f32, kind="ExternalInput")
    maskT = nc.dram_tensor("maskT", [P, SC], f32, kind="ExternalInput")
    oneC = nc.dram_tensor("oneC", [P, 1], bf16, kind="ExternalInput")
    out = nc.dram_tensor("out", [QB, D], f32, kind="ExternalOutput")

    with tile.TileContext(nc) as tc:
        with (
            tc.tile_pool(name="const", bufs=1) as constp,
            tc.tile_pool(name="big", bufs=DCH) as bigp,
            tc.tile_pool(name="wo", bufs=DCH) as wobigp,
            tc.tile_pool(name="w", bufs=2) as wpool,
            tc.tile_pool(name="kt", bufs=2) as ktpool,
            tc.tile_pool(name="va", bufs=SC) as vpool,
            tc.tile_pool(name="qtz", bufs=4) as qpool,
            tc.tile_pool(name="pt", bufs=4) as ptpool,
            tc.tile_pool(name="cat", bufs=1) as catp,
            tc.tile_pool(name="rr", bufs=1) as rpool,
            tc.tile_pool(name="osb", bufs=2) as outp,
            tc.tile_pool(name="pp", bufs=2, space="PSUM") as pp,
            tc.tile_pool(name="psc", bufs=2, space="PSUM") as psc,
            tc.tile_pool(name="po", bufs=2, space="PSUM") as pop,
        ):
            # ---- constants
            bq_sb = constp.tile([P, H // 2], f32, tag="bq")
            nc.sync.dma_start(out=bq_sb[:], in_=bq[:])
            bk_sb = constp.tile([P, H // 2], f32, tag="bk")
            nc.sync.dma_start(out=bk_sb[:], in_=bk[:])
            mask_sb = constp.tile([P, SC], f32, tag="mask")
            nc.sync.dma_start(out=mask_sb[:], in_=maskT[:])
            ones_sb = constp.tile([P, 1], bf16, tag="ones")
            nc.sync.dma_start(out=ones_sb[:], in_=oneC[:])
            bv_src = constp.tile([1, D], f32, tag="bvs")
            nc.sync.dma_start(out=bv_src[:], in_=bv[:])
            bo_src = constp.tile([1, D], f32, tag="bos")
            nc.sync.dma_start(out=bo_src[:], in_=bo[:])
            bv_rep = constp.tile([P, D], f32, tag="bvr")
            nc.gpsimd.partition_broadcast(bv_rep[:], bv_src[:])
            bo_rep = constp.tile([P, D], f32, tag="bor")
            nc.gpsimd.partition_broadcast(bo_rep[:], bo_src[:])

            for rep in range(UNROLL):
              concat = catp.tile([P, DCH, QB], bf16, tag="cat",
                                 name=f"cat{rep}")

              # ---- x^T resident in SBUF (8 chunks of [128, 2048])
              xt = []
              for d in range(DCH):
                  t = bigp.tile([P, S], bf16, tag="big", name=f"xt{rep}_{d}")
                  nc.sync.dma_start(out=t[:], in_=xT[d])
                  xt.append(t)

              NW = 4          # waves
              HPW = H // NW   # heads per wave

              for wave in range(NW):
                  groups = [2 * wave, 2 * wave + 1]
                  # ---- A: kT projection (2-head groups, output [2*64 dh, s])
                  kt = []
                  for gl, g in enumerate(groups):
                      wk_t = wpool.tile([P, DCH, P], bf16, tag="wk")
                      nc.sync.dma_start(out=wk_t[:], in_=wk[g])
                      ktile = ktpool.tile([P, S], f32r, tag="kt")
                      DR = 1 if "kv1" in ABLATE else DCH
                      for sb in range(4):
                          ps = pp.tile([P, 512], f32, tag="pp")
                          for d in range(DR):
                              nc.tensor.matmul(
                                  ps[:],
                                  wk_t[:, d, :],
                                  xt[d][:, sb * 512:(sb + 1) * 512],
                                  start=(d == 0),
                                  stop=(d == DR - 1),
                              )
                          nc.vector.tensor_scalar_add(
                              ktile[:, sb * 512:(sb + 1) * 512], ps[:],
                              bk_sb[:, g:g + 1],
                          )
                      kt.append(ktile)

                  # ---- A: v projection, 8 heads (this wave + the next) per
                  # matmul slot (N=512), emitted on even waves only.
                  if wave % 2 == 0:
                      wv_t = wpool.tile([P, DCH, 512], bf16, tag="wv")
                      nc.sync.dma_start(out=wv_t[:], in_=wv[wave // 2])
                      va8 = []
                      for sc in range(SC):
                          # masked keys are handled by the exp bias (-50 on
                          # masked partitions), so V needs no masking and no
                          # denominator column here.
                          vt = vpool.tile([P, 2 * HPW, 64], bf16, tag="va")
                          ps = pp.tile([P, 512], f32, tag="pp",
                                       name=f"vps_{wave}_{sc}")
                          DR = 1 if "kv1" in ABLATE else DCH
                          for d in range(DR):
                              nc.tensor.matmul(
                                  ps[:],
                                  xt[d][:, sc * P:(sc + 1) * P],
                                  wv_t[:, d, :],
                                  start=(d == 0),
                                  stop=(d == DR - 1),
                              )
                          ps_r = ps.rearrange("p (h e) -> p h e", e=64)
                          nc.vector.tensor_tensor(
                              vt[:],
                              ps_r,
                              bv_rep[:, wave * 256:(wave + 2) * 256].rearrange(
                                  "p (h e) -> p h e", e=64),
                              OP.add,
                          )
                          va8.append(vt)
                  voff = (wave % 2) * HPW

                  # ---- A: q projection for this wave's groups. One [128, 512]
                  # tile per group: rows 0:64 = even head, 64:128 = odd head
                  # (bias added, no zeroing -- the scores matmuls are row-tiled
                  # at K=64 so the other half is never read).
                  qtz = []
                  for gl, g in enumerate(groups):
                      wq_t = wpool.tile([P, DCH, P], bf16, tag="wq")
                      nc.sync.dma_start(out=wq_t[:], in_=wq[g])
                      ps = pp.tile([P, 512], f32, tag="pp")
                      for d in range(DCH):
                          nc.tensor.matmul(
                              ps[:],
                              wq_t[:, d, :],
                              xt[d][:, 0:QB],
                              start=(d == 0),
                              stop=(d == DCH - 1),
                          )
                      qz = qpool.tile([P, QB], f32r, tag="qtz")
                      nc.vector.tensor_scalar_add(
                          qz[:], ps[:], bq_sb[:, g:g + 1],
                      )
                      qtz.append(qz)

                  # ---- B: attention, one head PAIR at a time.
                  # scores: row-tiled (K=64 at row 0 / 64) -- one N=512 slot
                  #   computes both heads' scores for a chunk.
                  # AV: col-tiled (M=64 at col 0 / 64) -- one slot accumulates
                  #   both heads into one PSUM bank (h0 rows 0:64, h1 64:128).
                  # den: per chunk PAIR, four col-tiled M=1 matmuls with a
                  #   ones-column stationary -- one slot per 2 chunks covers
                  #   both heads' denominators.
                  for gl in range(2):
                      hl0, hl1 = 2 * gl, 2 * gl + 1
                      po_pair = pop.tile([P, QB], f32, tag="po")
                      den_ps = pop.tile([P, QB], f32, tag="po")
                      pts = {}

                      def emit_scores(sc):
                          # both heads' scores for chunk sc -> [128, 1024]
                          # psum (2 banks), one exp over both; key-pad mask is
                          # applied via the per-partition exp bias (0 / -50)
                          sps = psc.tile([P, 2, QB], f32, tag="ps")
                          nc.tensor.matmul(
                              sps[:, 0, :],
                              kt[gl][0:64, sc * P:(sc + 1) * P],
                              qtz[gl][0:64, :],
                              start=True,
                              stop=True,
                          )
                          nc.tensor.matmul(
                              sps[:, 1, :],
                              kt[gl][64:P, sc * P:(sc + 1) * P],
                              qtz[gl][64:P, :],
                              start=True,
                              stop=True,
                          )
                          pt = ptpool.tile([P, 2, QB], bf16, tag="pt")
                          if "exp_copy" in ABLATE:
                              nc.vector.tensor_scalar(
                                  pt[:], sps[:], 0.125, None, OP.mult)
                          else:
                              nc.scalar.activation(
                                  pt[:], sps[:], AF.Exp,
                                  bias=mask_sb[:, sc:sc + 1], scale=0.125,
                              )
                          pts[sc] = pt

                      def emit_av(sc):
                          pt = pts[sc]
                          nc.tensor.matmul(
                              po_pair[0:64, :],
                              va8[sc][:, voff + hl0, :],
                              pt[:, 0, :],
                              start=(sc == 0),
                              stop=(sc == SC - 1),
                          )
                          nc.tensor.matmul(
                              po_pair[64:P, :],
                              va8[sc][:, voff + hl1, :],
                              pt[:, 1, :],
                              start=(sc == 0),
                              stop=(sc == SC - 1),
                          )

                      def emit_den(sc):
                          # chunk pair (sc-1, sc): 4 M=1 col tiles at rows
                          # 0/32/64/96 of den_ps: [h0 even, h1 even, h0 odd,
                          # h1 odd]
                          pt_e, pt_o = pts.pop(sc - 1), pts.pop(sc)
                          pc = sc // 2
                          first, last = pc == 0, pc == SC // 2 - 1
                          for j, (ptx, h) in enumerate(
                              ((pt_e, 0), (pt_o, 0), (pt_e, 1), (pt_o, 1))
                          ):
                              r = (0, 64, 32, 96)[j]
                              nc.tensor.matmul(
                                  den_ps[r:r + 1, :],
                                  ones_sb[:],
                                  ptx[:, h, :],
                                  start=first,
                                  stop=last,
                                  tile_position=(0, r),
                              )

                      def emit_o(sc):
                          emit_av(sc)
                          if sc % 2 == 1:
                              emit_den(sc)

                      if "b1" in ABLATE:
                          emit_scores(0)
                          pt = pts.pop(0)
                          nc.tensor.matmul(
                              po_pair[0:64, :], va8[0][:, voff + hl0, :],
                              pt[:, 0, :],
                              start=True, stop=True)
                          nc.tensor.matmul(
                              po_pair[64:P, :], va8[0][:, voff + hl1, :],
                              pt[:, 1, :],
                              start=True, stop=True)
                      else:
                          emit_scores(0)
                          emit_scores(1)
                          for sc in range(2, SC):
                              emit_o(sc - 2)
                              emit_scores(sc)
                          emit_o(SC - 2)
                          emit_o(SC - 1)

                      # normalize: den_ps rows {0,64} = h0 even/odd partial
                      # denominators, {32,96} = h1. Collapse to one partition,
                      # one reciprocal + one broadcast for both heads.
                      cslot = wave * 2 + gl
                      dcp = rpool.tile([P, QB], f32, tag="dcp")
                      nc.vector.tensor_copy(dcp[0:97, :], den_ps[0:97, :])
                      # den4[h, j, :]: head h partials (even j=0, odd j=1)
                      den4 = rpool.tile([2, 2, QB], f32, tag="den4")
                      for (h, j), r in zip(
                          ((0, 0), (0, 1), (1, 0), (1, 1)), (0, 64, 32, 96)
                      ):
                          nc.sync.dma_start(
                              out=den4[h:h + 1, j, :], in_=dcp[r:r + 1, :])
                      dsum = rpool.tile([2, QB], f32, tag="dsum")
                      nc.vector.tensor_tensor(
                          dsum[:], den4[:, 0, :], den4[:, 1, :], OP.add)
                      dinv = rpool.tile([1, 2, QB], f32, tag="dinv")
                      nc.sync.dma_start(out=dinv[:], in_=dsum[:])
                      nc.vector.reciprocal(dinv[:], dinv[:])
                      rep = rpool.tile([P, 2, QB], f32, tag="rep")
                      nc.gpsimd.partition_broadcast(rep[:], dinv[0:1, :, :])
                      nc.vector.tensor_tensor(
                          concat[0:64, cslot, :], po_pair[0:64, :],
                          rep[0:64, 0, :], OP.mult,
                      )
                      nc.vector.tensor_tensor(
                          concat[64:P, cslot, :], po_pair[64:P, :],
                          rep[64:P, 1, :], OP.mult,
                      )

              # ---- C: output projection (contraction over h*dh in 8 chunks)
              wo_sb = []
              for c in range(DCH):
                  t = wobigp.tile([P, D], bf16, tag="wo")
                  nc.sync.dma_start(out=t[:], in_=woT[c])
                  wo_sb.append(t)
              for qt_i in range(QB // P):
                  for eb in range(2):
                      ps = pp.tile([P, 512], f32, tag="pp")
                      CR = 1 if "c1" in ABLATE else DCH
                      for c in range(CR):
                          nc.tensor.matmul(
                              ps[:],
                              concat[:, c, qt_i * P:(qt_i + 1) * P],
                              wo_sb[c][:, eb * 512:(eb + 1) * 512],
                              start=(c == 0),
                              stop=(c == CR - 1),
                          )
                      osb = outp.tile([P, 512], f32, tag="osb")
                      nc.vector.tensor_tensor(
                          osb[:], ps[:], bo_rep[:, eb * 512:(eb + 1) * 512],
                          OP.add,
                      )
                      nc.sync.dma_start(
                          out=out[qt_i * P:(qt_i + 1) * P,
                                  eb * 512:(eb + 1) * 512],
                          in_=osb[:],
                      )

    nc.compile()
    nc.finalize()
    return nc


def _round_fp32r(a):
    """Round fp32 values to fp32r (TF32-like, 11-bit mantissa, RNE)."""
    u = np.ascontiguousarray(a, dtype=np.float32).view(np.uint32).astype(np.uint64)
    r = ((u + 0x7FF + ((u >> 12) & 1)) & 0xFFFFF000).astype(np.uint32)
    return r.view(np.float32).reshape(a.shape)


def prep_inputs(x, pad_mask, wq, wk, wv, bq, bk, bv, wo, bo):
    """Build per-core input maps (host-side shard + layout prep)."""
    import ml_dtypes

    bf16 = ml_dtypes.bfloat16
    x = np.ascontiguousarray(np.asarray(x, dtype=np.float32))
    pad_mask = np.asarray(pad_mask)
    wq = np.asarray(wq, dtype=np.float32)
    wk = np.asarray(wk, dtype=np.float32)
    wv = np.asarray(wv, dtype=np.float32)
    bq = np.asarray(bq, dtype=np.float32)
    bk = np.asarray(bk, dtype=np.float32)
    bv = np.asarray(bv, dtype=np.float32)
    wo = np.asarray(wo, dtype=np.float32)
    bo = np.asarray(bo, dtype=np.float32)

    # weights: [H, D, DH] -> [d, h*dh] (h-major columns)
    def stack_groups(w, gsz):
        ws = np.ascontiguousarray(w.transpose(1, 0, 2).reshape(D, D))
        # -> [group, di, do, gsz*DH]
        m = gsz * DH
        arr = ws.reshape(DCH, P, H // gsz, m).transpose(2, 1, 0, 3)
        return np.ascontiguousarray(arr)

    wq_dev = stack_groups(wq, 2).astype(bf16)
    wk_dev = stack_groups(wk, 2).astype(bf16)
    wv_dev = stack_groups(wv, 8).astype(bf16)
    woT_dev = np.ascontiguousarray(wo.T).reshape(DCH, P, D).astype(bf16)
    bq_dev = np.ascontiguousarray(bq.reshape(H // 2, P).T)
    bk_dev = np.ascontiguousarray(bk.reshape(H // 2, P).T)
    bv_dev = np.ascontiguousarray(bv.reshape(1, D))
    bo_dev = np.ascontiguousarray(bo.reshape(1, D))

    in_maps = []
    for c in range(NCORES):
        b, qo = c // CPB, c % CPB
        # transpose + roll the s axis so this core's q rows are cols 0:QB
        xt = x[b].T  # [D, S]
        xt = np.roll(xt, -qo * QB, axis=1)
        xt_dev = np.ascontiguousarray(xt).astype(bf16).reshape(DCH, P, S)
        # exp-bias mask: 0 for valid keys, -50 for padded (exp -> ~0)
        mb = np.where(pad_mask[b] != 0, 0.0, -50.0).astype(np.float32)
        mb = np.roll(mb, -qo * QB)
        maskT_dev = np.ascontiguousarray(mb.reshape(SC, P).T)
        in_maps.append({
            "xT": xt_dev, "wq": wq_dev, "wk": wk_dev, "wv": wv_dev,
            "woT": woT_dev, "bq": bq_dev, "bk": bk_dev, "bv": bv_dev,
            "bo": bo_dev, "maskT": maskT_dev,
            "oneC": np.ones((P, 1), dtype=bf16),
        })
    return in_maps


def kernel(**inputs):
    global LAST_RESULTS
    from concourse.bass_utils import run_bass_kernel_spmd

    if "nc" not in _compiled:
        _compiled["nc"] = _build_program()
    nc = _compiled["nc"]

    in_maps = prep_inputs(**inputs)
    res = run_bass_kernel_spmd(
        nc, in_maps, list(range(NCORES)),
        trace=bool(os.environ.get("BASS_TRACE")),
    )
    LAST_RESULTS = res

    out = np.empty((B, S, D), dtype=np.float32)
    for c in range(NCORES):
        b, qo = c // CPB, c % CPB
        out[b, qo * QB:(qo + 1) * QB, :] = res.results[c]["out"]
    return out

